# revision 22
# baseline (speedup 1.0000x reference)
"""Trainium2 Bass kernel: PSI block (LN1 -> sigmoid-gated value -> chunked
normalized cumsum -> residual -> LN2 -> exact-gelu FFN -> residual).

Sharding: 32768 tokens split into 8 contiguous 4096-token shards (chunk- and
batch-boundary aligned), one per NeuronCore; dim-sized weights replicated.

Fully fused single pass per 512-token macro: LN1 stats, z.T via PE
transposes, bf16 gate/value matmuls, sigmoid, chunked cumsum via
block-triangular matmul, x2 = x + mem kept SBUF-resident (no DRAM round
trip), LN2, FFN1 (f-block stationary, exact gelu via Erf), FFN2
token-stationary so the output lands in natural [tok, d] layout with the
fp32 residual folded in by the DVE drain. Pass-A work for macro m+1 is
interleaved into the FFN1 f-loop of macro m so DVE/ACT work hides under
PE work.
"""

import sys

sys.path.insert(0, "/opt/trn_rl_repo")

import numpy as np
import ml_dtypes
from contextlib import ExitStack

B, S, D, CHUNK = 4, 8192, 768, 64
NCORES = 8
TOTAL = B * S              # 32768 tokens
TPC = TOTAL // NCORES      # 4096 tokens per core
KD = D // 128              # 6 k-blocks over D
H = 4 * D                  # 3072 FFN hidden
KH = H // 128              # 24 k-blocks over H
MACRO = 512                # token macro
INV_SQRT2 = 0.7071067811865476


def build(T=TPC, erf_ok=True, gbias=False, vbias=False, f1bias=False, f2bias=False,
          reps=1):
    import concourse.bass as bass
    import concourse.bacc as bacc
    import concourse.tile as tile
    from concourse import mybir

    F32 = mybir.dt.float32
    BF16 = mybir.dt.bfloat16
    I32 = mybir.dt.int32
    AF = mybir.ActivationFunctionType
    ALU = mybir.AluOpType
    PSUM = bass.MemorySpace.PSUM
    ts = bass.ts

    NT = T // 128
    NM = T // MACRO
    NS = MACRO // 128
    gv_b = gbias or vbias
    any_bias = gv_b or f1bias or f2bias

    nc = bacc.Bacc(None, target_bir_lowering=False, debug=False)

    x_d = nc.dram_tensor("x", [T, D], F32, kind="ExternalInput")
    wgv_d = nc.dram_tensor("wgv", [128, KD, 2 * D], BF16, kind="ExternalInput")
    w1_d = nc.dram_tensor("w1", [128, KD, H], BF16, kind="ExternalInput")
    w2_d = nc.dram_tensor("w2", [128, KH, D], BF16, kind="ExternalInput")
    u_d = nc.dram_tensor("u", [128, 128], BF16, kind="ExternalInput")
    idb_d = nc.dram_tensor("idb", [128, 128], BF16, kind="ExternalInput")
    bgv_d = nc.dram_tensor("bgv", [1, 2 * D], BF16, kind="ExternalInput") if gv_b else None
    b1_d = nc.dram_tensor("b1", [1, H], BF16, kind="ExternalInput") if f1bias else None
    b2_d = nc.dram_tensor("b2", [1, D], BF16, kind="ExternalInput") if f2bias else None
    out_d = nc.dram_tensor("out", [T, D], F32, kind="ExternalOutput")

    with tile.TileContext(nc) as tc, ExitStack() as ctx:
        const = ctx.enter_context(tc.tile_pool(name="const", bufs=1))
        pa = ctx.enter_context(tc.tile_pool(name="pa", bufs=1))
        psa = ctx.enter_context(tc.tile_pool(name="psa", bufs=1, space=PSUM))

        # x-tile DMAs for macro 0 ahead of everything (first LN1 stats gate
        # the whole pipeline); weight DMAs in few big chunks to keep the
        # Sync-queue issue cost (~0.65us per dma_start) off the critical path
        xs, hus, lnts, x2s = {}, {}, {}, {}

        def stage_xdma(t):
            x_sb = pa.tile([128, D], F32, tag="x", bufs=4, name="x_sb")
            nc.sync.dma_start(x_sb[:], x_d[128 * t:128 * (t + 1), :])
            xs[t] = x_sb

        u_sb = const.tile([128, 128], BF16, tag="u")
        nc.sync.dma_start(u_sb[:], u_d[:])
        idb_sb = const.tile([128, 128], BF16, tag="idb")
        nc.sync.dma_start(idb_sb[:], idb_d[:])
        for t in range(min(NS, NT)):
            stage_xdma(t)

        wgv_sb = const.tile([128, KD, 2 * D], BF16, tag="wgv")
        for k in range(0, KD, 3):
            nc.sync.dma_start(wgv_sb[:, k:k + 3, :], wgv_d[:, k:k + 3, :])
        eps_sb = const.tile([128, 1], F32, tag="eps")
        nc.vector.memset(eps_sb[:], 1e-6)
        if gv_b:
            bgv_sb = const.tile([1, 2 * D], BF16, tag="bgv")
            nc.sync.dma_start(bgv_sb[:], bgv_d[:])
        if f1bias:
            b1_sb = const.tile([1, H], BF16, tag="b1")
            nc.sync.dma_start(b1_sb[:], b1_d[:])
        if f2bias:
            b2_sb = const.tile([1, D], BF16, tag="b2")
            nc.sync.dma_start(b2_sb[:], b2_d[:])
        if any_bias:
            ones_sb = const.tile([1, MACRO], BF16, tag="ones")
            nc.vector.memset(ones_sb[:], 1.0)

        w1_sb = const.tile([128, KD, H], BF16, tag="w1")
        for k in range(0, KD, 2):
            nc.sync.dma_start(w1_sb[:, k:k + 2, :], w1_d[:, k:k + 2, :])
        w2_sb = const.tile([128, KH, D], BF16, tag="w2")
        for k in range(0, KH, 8):
            nc.sync.dma_start(w2_sb[:, k:k + 8, :], w2_d[:, k:k + 8, :])

        # h2T for a whole macro, double-buffered; uT for the full FFN hidden
        h2t_tiles = [pa.tile([128, KD, MACRO], BF16, tag="h2T", bufs=2,
                             name="h2T") for _ in range(2)]
        uT = const.tile([128, KH, MACRO], BF16, tag="uT")

        def ln_stats(tag, src):
            """Row stats of src [128, D] f32: returns (nmu, v) = (-mean, var+1e-5)."""
            sqscr = pa.tile([128, D], BF16, tag="sqscr", bufs=2, name="sqscr")
            sqs = pa.tile([128, 1], F32, tag=tag + "_sqs", bufs=2, name="sqs")
            nc.scalar.activation(sqscr[:], src[:], AF.Square, accum_out=sqs[:])
            xsum = pa.tile([128, 1], F32, tag=tag + "_xs", bufs=2, name="xs")
            nc.vector.tensor_reduce(xsum[:], src[:], mybir.AxisListType.X, ALU.add)
            nmu = pa.tile([128, 1], F32, tag=tag + "_nmu", bufs=2, name="nmu")
            nc.vector.tensor_scalar(nmu[:], xsum[:], -1.0 / D, None, op0=ALU.mult)
            v = pa.tile([128, 1], F32, tag=tag + "_v", bufs=2, name="v")
            nc.vector.tensor_scalar(v[:], sqs[:], 1.0 / D, 1e-5, op0=ALU.mult, op1=ALU.add)
            m2 = pa.tile([128, 1], F32, tag=tag + "_m2", bufs=2, name="m2")
            nc.vector.tensor_mul(m2[:], nmu[:], nmu[:])
            nc.vector.tensor_sub(v[:], v[:], m2[:])
            return nmu, v

        def newton_rsqrt(tag, v):
            """y ~ rsqrt(v) for v [128,1] f32 > 0; quake seed + 2 NR iters on DVE."""
            y = pa.tile([128, 1], F32, tag=tag + "_y", bufs=2, name="y")
            a = pa.tile([128, 1], F32, tag=tag + "_a", bufs=2, name="a")
            nc.vector.tensor_scalar(
                y[:].bitcast(I32), v[:].bitcast(I32), 1, -1,
                op0=ALU.logical_shift_right, op1=ALU.bitwise_xor,
            )
            nc.vector.tensor_scalar(
                y[:].bitcast(I32), y[:].bitcast(I32), 0x5F3759E0, None, op0=ALU.add
            )
            for it in range(2):
                nc.vector.tensor_mul(a[:], y[:], y[:])
                nc.vector.tensor_mul(a[:], a[:], v[:])
                nc.vector.tensor_scalar(a[:], a[:], -0.5, 1.5, op0=ALU.mult, op1=ALU.add)
                nc.vector.tensor_mul(y[:], y[:], a[:])
            return y

        # ---- pass-A chunks for one 128-token tile ----
        def c1_stats(t):
            x_sb = xs[t]
            nmu, v = ln_stats("s1", x_sb)
            rstd = newton_rsqrt("n1", v)
            nmr1 = pa.tile([128, 1], F32, tag="nmr1", bufs=2, name="nmr1")
            nc.vector.tensor_mul(nmr1[:], nmu[:], rstd[:])
            hu = pa.tile([128, D], BF16, tag="hu", bufs=2, name="hu")
            nc.vector.tensor_scalar(hu[:], x_sb[:], rstd[:], nmr1[:],
                                    op0=ALU.mult, op1=ALU.add)
            hus[t] = hu

        def c2_lnT(t):
            hu = hus.pop(t)
            trps = psa.tile([128, KD, 128], BF16, tag="trps", bufs=1,
                            padded_shape=[128, 8, 128], name="trps")
            for k in range(KD):
                nc.tensor.transpose(trps[:, k, :], hu[:, ts(k, 128)], idb_sb[:])
            lnT = pa.tile([128, KD, 128], BF16, tag="lnT", bufs=2, name="lnT")
            nc.scalar.copy(lnT[:], trps[:])
            lnts[t] = lnT

        def c3_gv(t):
            lnT = lnts.pop(t)
            pgv = [None] * 3
            for bb in range(3):
                pgv[bb] = psa.tile([128, 512], F32, tag="gvps", bufs=3, name="pgv")
                mm = [(lnT[:, k, :], wgv_sb[:, k, 512 * bb:512 * (bb + 1)])
                      for k in range(KD)]
                if gv_b:
                    mm.append((ones_sb[0:1, 0:128],
                               bgv_sb[0:1, 512 * bb:512 * (bb + 1)]))
                for i, (l, r) in enumerate(mm):
                    nc.tensor.matmul(pgv[bb][:], l, r,
                                     start=(i == 0), stop=(i == len(mm) - 1))
            # gvg packs [g 0:768 | g*v 0:768]
            gvg = pa.tile([128, 2 * D], BF16, tag="gvg", bufs=2, name="gvg")
            nc.scalar.activation(gvg[:, 0:512], pgv[0][:], AF.Sigmoid)
            nc.scalar.activation(gvg[:, 512:768], pgv[2][:, 0:256], AF.Sigmoid)
            nc.vector.tensor_mul(gvg[:, 768:1280], gvg[:, 0:512], pgv[1][:])
            nc.vector.tensor_mul(gvg[:, 1280:1536], gvg[:, 512:768],
                                 pgv[2][:, 256:512])
            return gvg

        def c4a_cs(t, gvg):
            cs = [None] * 3
            for bb in range(3):
                cs[bb] = psa.tile([128, 512], F32, tag="gvps", bufs=3, name="cs")
                nc.tensor.matmul(cs[bb][:], u_sb[:],
                                 gvg[:, 512 * bb:512 * (bb + 1)],
                                 start=True, stop=True)
            den = pa.tile([128, D], F32, tag="den", bufs=2, name="den")
            mem = pa.tile([128, D], F32, tag="mem", bufs=2, name="mem")
            nc.scalar.activation(den[:, 0:512], cs[0][:], AF.Identity,
                                 bias=eps_sb[:])
            nc.scalar.activation(den[:, 512:768], cs[1][:, 0:256], AF.Identity,
                                 bias=eps_sb[:])
            nc.vector.reciprocal_approx_fast(den[:], den[:])
            nc.vector.tensor_mul(mem[:, 0:256], den[:, 0:256], cs[1][:, 256:512])
            nc.vector.tensor_mul(mem[:, 256:768], den[:, 256:768], cs[2][:])
            x_sb = xs.pop(t)
            x2 = pa.tile([128, D], F32, tag="x2", bufs=2 * NS, name="x2")
            nc.vector.tensor_add(x2[:], x_sb[:], mem[:])
            x2s[t] = x2

        def c4b_ln2(t):
            x2 = x2s[t]
            nmu2, v2 = ln_stats("s2", x2)
            rstd2 = newton_rsqrt("n2", v2)
            nmr2 = pa.tile([128, 1], F32, tag="nmr2", bufs=2, name="nmr2")
            nc.vector.tensor_mul(nmr2[:], nmu2[:], rstd2[:])
            h2s = pa.tile([128, D], BF16, tag="h2s", bufs=2, name="h2s")
            nc.vector.tensor_scalar(h2s[:], x2[:], rstd2[:], nmr2[:],
                                    op0=ALU.mult, op1=ALU.add)
            hus[("h2", t)] = h2s

        def c5_h2T(t, h2t_sb, s):
            h2s = hus.pop(("h2", t))
            trps = psa.tile([128, KD, 128], BF16, tag="trps", bufs=1,
                            padded_shape=[128, 8, 128], name="trps")
            for k in range(KD):
                nc.tensor.transpose(trps[:, k, :], h2s[:, ts(k, 128)], idb_sb[:])
            nc.scalar.copy(h2t_sb[:, :, 128 * s:128 * (s + 1)], trps[:])

        def tile_chunks(t, h2t_sb, s):
            gvg_box = []
            return [
                lambda: c1_stats(t),
                lambda: c2_lnT(t),
                lambda: gvg_box.append(c3_gv(t)),
                lambda: c4a_cs(t, gvg_box.pop()),
                lambda: c4b_ln2(t),
                lambda: c5_h2T(t, h2t_sb, s),
            ]

        def macro_chunks(m, h2t_sb):
            """Interleave order for the 4 tiles of macro m: stats for all
            tiles first, then the gv/cumsum chains, transposes last — gives
            every cross-engine dependency >= 1 FFN1 f-block of headroom."""
            percall = [tile_chunks(m * NS + s, h2t_sb, s) for s in range(NS)]
            order = [c[0] for c in percall]
            for c in percall:
                order += c[1:5]
            order += [c[5] for c in percall]
            return order

        # ---- FFN for one macro (512 tokens), with interleaved chunks ----
        def emit_ffn(m, h2t_sb, chunks):
            tok0 = MACRO * m
            ci = iter(chunks)
            # FFN1: f-block stationary -> uT[f] = gelu-ish in [f, tok] layout
            for f in range(KH):
                pT = psa.tile([128, MACRO], F32, tag="pT", bufs=2, name="pT")
                mm = [(w1_sb[:, k, 128 * f:128 * (f + 1)], h2t_sb[:, k, :])
                      for k in range(KD)]
                if f1bias:
                    mm.append((b1_sb[0:1, 128 * f:128 * (f + 1)],
                               ones_sb[0:1, 0:MACRO]))
                for i, (l, r) in enumerate(mm):
                    nc.tensor.matmul(pT[:], l, r,
                                     start=(i == 0), stop=(i == len(mm) - 1))
                e_sb = pa.tile([128, MACRO], BF16, tag="e", bufs=2, name="e_sb")
                nc.scalar.activation(e_sb[:], pT[:],
                                     AF.Erf if erf_ok else AF.Tanh,
                                     scale=INV_SQRT2)
                nc.vector.scalar_tensor_tensor(uT[:, f, :], e_sb[:], 1.0, pT[:],
                                               op0=ALU.add, op1=ALU.mult)
                nxt = next(ci, None)
                if nxt is not None:
                    nxt()
            for nxt in ci:
                nxt()
            # FFN2: token-stationary; output in natural [tok, d] layout.
            # Per token-block: 512-wide half then 256-wide half in separate
            # single-buffered banks so each drain overlaps the other half's
            # (or the next block's) matmuls.
            for tb in range(NS):
                t = m * NS + tb
                x2 = x2s.pop(t)
                last = (m == NM - 1 and tb == NS - 1)
                osb = pa.tile([128, D], F32, tag="osb", bufs=3, name="osb")
                for tag, off, ncols in (("outk0", 0, 512), ("outk1", 512, 256)):
                    ok = psa.tile([128, 512], F32, tag=tag, bufs=1, name=tag)
                    nmm = KH + (1 if f2bias else 0)
                    for f in range(KH):
                        nc.tensor.matmul(ok[:, 0:ncols],
                                         uT[:, f, 128 * tb:128 * (tb + 1)],
                                         w2_sb[:, f, off:off + ncols],
                                         start=(f == 0), stop=(f == nmm - 1))
                    if f2bias:
                        nc.tensor.matmul(ok[:, 0:ncols], ones_sb[0:1, 0:128],
                                         b2_sb[0:1, off:off + ncols],
                                         start=False, stop=True)
                    nc.vector.tensor_add(osb[:, off:off + ncols], ok[:, 0:ncols],
                                         x2[:, off:off + ncols])
                    if last:
                        # tail trim: ship each half as soon as it drains
                        nc.sync.dma_start(
                            out_d[tok0 + 128 * tb:tok0 + 128 * (tb + 1),
                                  off:off + ncols],
                            osb[:, off:off + ncols])
                if not last:
                    nc.sync.dma_start(
                        out_d[tok0 + 128 * tb:tok0 + 128 * (tb + 1), :], osb[:])

        for _ in range(reps):
            # prologue: macro 0 pass A, diagonally pipelined
            chunks0 = [tile_chunks(t, h2t_tiles[0], t) for t in range(NS)]
            for wave in range(6 + 2 * (NS - 1)):
                for s in range(NS):
                    k = wave - 2 * s
                    if 0 <= k < 6:
                        chunks0[s][k]()
            for m in range(NM):
                if m + 1 < NM:
                    for s in range(NS):
                        stage_xdma((m + 1) * NS + s)
                    nxt_chunks = macro_chunks(m + 1, h2t_tiles[(m + 1) % 2])
                else:
                    nxt_chunks = []
                emit_ffn(m, h2t_tiles[m % 2], nxt_chunks)

    nc.compile()
    return nc


def _fold(inputs):
    f32 = np.float32
    bf16 = ml_dtypes.bfloat16
    n1w = np.asarray(inputs["norm1_w"], f32)
    n1b = np.asarray(inputs["norm1_b"], f32)
    n2w = np.asarray(inputs["norm2_w"], f32)
    n2b = np.asarray(inputs["norm2_b"], f32)
    gW = np.asarray(inputs["gate_W"], f32)
    gb = np.asarray(inputs["gate_b"], f32)
    vW = np.asarray(inputs["value_W"], f32)
    vb = np.asarray(inputs["value_b"], f32)
    W1 = np.asarray(inputs["ffn_W1"], f32)
    b1 = np.asarray(inputs["ffn_b1"], f32)
    W2 = np.asarray(inputs["ffn_W2"], f32)
    b2 = np.asarray(inputs["ffn_b2"], f32)

    bg = (n1b @ gW + gb).astype(bf16).reshape(1, D)
    bv = (n1b @ vW + vb).astype(bf16).reshape(1, D)
    b1f = (n2b @ W1 + b1).astype(bf16).reshape(1, H)
    b2f = b2.astype(bf16).reshape(1, D)
    flags = (bool(bg.any()), bool(bv.any()), bool(b1f.any()), bool(b2f.any()))

    tri = np.triu(np.ones((CHUNK, CHUNK), f32))
    u = np.zeros((128, 128), f32)
    for c in range(128 // CHUNK):
        u[c * CHUNK:(c + 1) * CHUNK, c * CHUNK:(c + 1) * CHUNK] = tri

    gWs = n1w[:, None] * gW
    vWs = n1w[:, None] * vW
    wgv = np.concatenate(
        [gWs[:, 0:512], vWs[:, 0:512], gWs[:, 512:768], vWs[:, 512:768]], axis=1)
    arrs = {
        "wgv": np.ascontiguousarray(
            wgv.reshape(KD, 128, 2 * D).transpose(1, 0, 2).astype(bf16)),
        "w1": np.ascontiguousarray(
            (n2w[:, None] * W1).reshape(KD, 128, H).transpose(1, 0, 2).astype(bf16)),
        "w2": np.ascontiguousarray(
            (0.5 * W2).reshape(KH, 128, D).transpose(1, 0, 2).astype(bf16)),
        "u": u.astype(bf16),
        "idb": np.eye(128, dtype=bf16),
    }
    if flags[0] or flags[1]:
        arrs["bgv"] = np.concatenate(
            [bg[:, 0:512], bv[:, 0:512], bg[:, 512:768], bv[:, 512:768]], axis=1)
    if flags[2]:
        arrs["b1"] = b1f
    if flags[3]:
        arrs["b2"] = b2f
    return arrs, flags


_CACHE: dict = {}


def _get_exec(flags):
    """Build (once) the Bass module and a cached jitted PJRT executable."""
    if _CACHE.get("flags") == flags:
        return _CACHE
    import jax
    from concourse import mybir
    from concourse.bass2jax import (
        Mesh, PartitionSpec, shard_map, _bass_exec_p, install_neuronx_cc_hook,
        partition_id_tensor,
    )

    nc = build(TPC, True, *flags)
    install_neuronx_cc_hook()
    assert nc.dbg_addr is None
    partition_name = nc.partition_id_tensor.name if nc.partition_id_tensor else None

    in_names, out_names, out_avals, zero_outs = [], [], [], []
    for alloc in nc.m.functions[0].allocations:
        if not isinstance(alloc, mybir.MemoryLocationSet):
            continue
        name = alloc.memorylocations[0].name
        if alloc.kind == "ExternalInput":
            if name != partition_name:
                in_names.append(name)
        elif alloc.kind == "ExternalOutput":
            shape = tuple(alloc.tensor_shape)
            dtype = mybir.dt.np(alloc.dtype)
            out_names.append(name)
            out_avals.append(jax.core.ShapedArray(shape, dtype))
            zero_outs.append(np.zeros(shape, dtype))
    n_params = len(in_names)
    n_outs = len(out_avals)
    all_names = in_names + out_names
    if partition_name is not None:
        all_names = all_names + [partition_name]
    donate = tuple(range(n_params, n_params + n_outs))

    def _body(*args):
        operands = list(args)
        if partition_name is not None:
            operands.append(partition_id_tensor())
        outs = _bass_exec_p.bind(
            *operands,
            out_avals=tuple(out_avals),
            in_names=tuple(all_names),
            out_names=tuple(out_names),
            lowering_input_output_aliases=(),
            sim_require_finite=True,
            sim_require_nnan=True,
            nc=nc,
        )
        return tuple(outs)

    devices = jax.devices()[:NCORES]
    assert len(devices) == NCORES
    mesh = Mesh(np.asarray(devices), ("core",))
    sharded = jax.jit(
        shard_map(_body, mesh=mesh, in_specs=(PartitionSpec("core"),) * (n_params + n_outs),
                  out_specs=(PartitionSpec("core"),) * n_outs, check_rep=False),
        donate_argnums=donate, keep_unused=True,
    )
    _CACHE.clear()
    _CACHE.update(
        flags=flags, nc=nc, sharded=sharded, in_names=in_names,
        out_names=out_names, out_avals=out_avals, zero_outs=zero_outs, mesh=mesh,
    )
    return _CACHE


def _run(arrs, flags, x_flat):
    st = _get_exec(flags)
    concat_in = []
    for name in st["in_names"]:
        if name == "x":
            concat_in.append(np.ascontiguousarray(x_flat))
        else:
            a = arrs[name]
            concat_in.append(np.concatenate([a] * NCORES, axis=0))
    concat_zeros = [
        np.zeros((NCORES * z.shape[0], *z.shape[1:]), z.dtype) for z in st["zero_outs"]
    ]
    out_arrs = st["sharded"](*concat_in, *concat_zeros)
    i = st["out_names"].index("out")
    return np.asarray(out_arrs[i])


def _assemble(results):
    """Full [B,S,D] output from per-core result dicts."""
    parts = [np.asarray(results[c]["out"]) for c in range(NCORES)]
    return np.concatenate(parts, axis=0).reshape(B, S, D).astype(np.float32)


def kernel(**inputs):
    x = np.asarray(inputs["x"], np.float32).reshape(TOTAL, D)
    arrs, flags = _fold(inputs)
    try:
        o = _run(arrs, flags, x)
        return np.asarray(o).reshape(B, S, D).astype(np.float32)
    except Exception:
        from concourse.bass_utils import run_bass_kernel_spmd
        if _CACHE.get("flags") != flags or "nc" not in _CACHE:
            _CACHE.clear()
            _CACHE["nc"] = build(TPC, True, *flags)
            _CACHE["flags"] = flags
        in_maps = [
            {**arrs, "x": np.ascontiguousarray(x[c * TPC:(c + 1) * TPC])}
            for c in range(NCORES)
        ]
        res = run_bass_kernel_spmd(_CACHE["nc"], in_maps, list(range(NCORES)),
                                   trace=False)
        return _assemble(res.results)


# revision 28
# speedup vs baseline: 1.0151x; 1.0151x over previous
"""Trainium2 Bass kernel: PSI block (LN1 -> sigmoid-gated value -> chunked
normalized cumsum -> residual -> LN2 -> exact-gelu FFN -> residual).

Sharding: 32768 tokens split into 8 contiguous 4096-token shards (chunk- and
batch-boundary aligned), one per NeuronCore; dim-sized weights replicated.

Fully fused single pass per 512-token macro: LN1 stats, z.T via PE
transposes, bf16 gate/value matmuls, sigmoid, chunked cumsum via
block-triangular matmul, x2 = x + mem kept SBUF-resident (no DRAM round
trip), LN2, FFN1 (f-block stationary, exact gelu via Erf), FFN2
token-stationary so the output lands in natural [tok, d] layout with the
fp32 residual folded in by the DVE drain. Pass-A work for macro m+1 is
interleaved into the FFN1 f-loop of macro m so DVE/ACT work hides under
PE work.
"""

import sys

sys.path.insert(0, "/opt/trn_rl_repo")

import numpy as np
import ml_dtypes
from contextlib import ExitStack

B, S, D, CHUNK = 4, 8192, 768, 64
NCORES = 8
TOTAL = B * S              # 32768 tokens
TPC = TOTAL // NCORES      # 4096 tokens per core
KD = D // 128              # 6 k-blocks over D
H = 4 * D                  # 3072 FFN hidden
KH = H // 128              # 24 k-blocks over H
MACRO = 512                # token macro
INV_SQRT2 = 0.7071067811865476


def build(T=TPC, erf_ok=True, gbias=False, vbias=False, f1bias=False, f2bias=False,
          reps=1):
    import concourse.bass as bass
    import concourse.bacc as bacc
    import concourse.tile as tile
    from concourse import mybir

    F32 = mybir.dt.float32
    BF16 = mybir.dt.bfloat16
    I32 = mybir.dt.int32
    AF = mybir.ActivationFunctionType
    ALU = mybir.AluOpType
    PSUM = bass.MemorySpace.PSUM
    ts = bass.ts

    NT = T // 128
    NM = T // MACRO
    NS = MACRO // 128
    gv_b = gbias or vbias
    any_bias = gv_b or f1bias or f2bias

    nc = bacc.Bacc(None, target_bir_lowering=False, debug=False)

    x_d = nc.dram_tensor("x", [T, D], F32, kind="ExternalInput")
    wgv_d = nc.dram_tensor("wgv", [128, KD, 2 * D], BF16, kind="ExternalInput")
    w1_d = nc.dram_tensor("w1", [128, KD, H], BF16, kind="ExternalInput")
    w2_d = nc.dram_tensor("w2", [128, KH, D], BF16, kind="ExternalInput")
    u_d = nc.dram_tensor("u", [128, 128], BF16, kind="ExternalInput")
    idb_d = nc.dram_tensor("idb", [128, 128], BF16, kind="ExternalInput")
    bgv_d = nc.dram_tensor("bgv", [1, 2 * D], BF16, kind="ExternalInput") if gv_b else None
    b1_d = nc.dram_tensor("b1", [1, H], BF16, kind="ExternalInput") if f1bias else None
    b2_d = nc.dram_tensor("b2", [1, D], BF16, kind="ExternalInput") if f2bias else None
    out_d = nc.dram_tensor("out", [T, D], F32, kind="ExternalOutput")

    with tile.TileContext(nc) as tc, ExitStack() as ctx:
        const = ctx.enter_context(tc.tile_pool(name="const", bufs=1))
        pa = ctx.enter_context(tc.tile_pool(name="pa", bufs=1))
        psa = ctx.enter_context(tc.tile_pool(name="psa", bufs=1, space=PSUM))

        # x-tile DMAs for macro 0 ahead of everything (first LN1 stats gate
        # the whole pipeline); weight DMAs in few big chunks to keep the
        # Sync-queue issue cost (~0.65us per dma_start) off the critical path
        xs, hus, lnts, x2s = {}, {}, {}, {}

        def stage_xdma(t):
            x_sb = pa.tile([128, D], F32, tag="x", bufs=8, name="x_sb")
            nc.sync.dma_start(x_sb[:], x_d[128 * t:128 * (t + 1), :])
            xs[t] = x_sb

        u_sb = const.tile([128, 128], BF16, tag="u")
        nc.sync.dma_start(u_sb[:], u_d[:])
        idb_sb = const.tile([128, 128], BF16, tag="idb")
        nc.sync.dma_start(idb_sb[:], idb_d[:])
        for t in range(min(NS, NT)):
            stage_xdma(t)

        # weights are loaded in chunks along their CONSUMPTION axis so each
        # consumer starts as soon as its first chunk lands: wgv by bb-column
        # group (c3 does bb=0,1,2 in order), w1 by f-column group (FFN1 goes
        # f ascending), w2 by output-column half (FFN2 does [0:512] first)
        wgv_sb = const.tile([128, KD, 2 * D], BF16, tag="wgv")
        for c in range(0, 2 * D, 512):
            nc.sync.dma_start(wgv_sb[:, :, c:c + 512], wgv_d[:, :, c:c + 512])
        eps_sb = const.tile([128, 1], F32, tag="eps")
        nc.vector.memset(eps_sb[:], 1e-6)
        if gv_b:
            bgv_sb = const.tile([1, 2 * D], BF16, tag="bgv")
            nc.sync.dma_start(bgv_sb[:], bgv_d[:])
        if f1bias:
            b1_sb = const.tile([1, H], BF16, tag="b1")
            nc.sync.dma_start(b1_sb[:], b1_d[:])
        if f2bias:
            b2_sb = const.tile([1, D], BF16, tag="b2")
            nc.sync.dma_start(b2_sb[:], b2_d[:])
        if any_bias:
            ones_sb = const.tile([1, MACRO], BF16, tag="ones")
            nc.vector.memset(ones_sb[:], 1.0)

        w1_sb = const.tile([128, KD, H], BF16, tag="w1")
        for c in range(0, H, 1024):
            nc.sync.dma_start(w1_sb[:, :, c:c + 1024], w1_d[:, :, c:c + 1024])
        # macro-1 x tiles must not queue behind w2 (needed ~25us before it)
        for t in range(NS, min(2 * NS, NT)):
            stage_xdma(t)
        w2_sb = const.tile([128, KH, D], BF16, tag="w2")
        nc.sync.dma_start(w2_sb[:, :, 0:512], w2_d[:, :, 0:512])
        nc.sync.dma_start(w2_sb[:, :, 512:768], w2_d[:, :, 512:768])

        # h2T for a whole macro, double-buffered; uT for the full FFN hidden
        h2t_tiles = [pa.tile([128, KD, MACRO], BF16, tag="h2T", bufs=2,
                             name="h2T") for _ in range(2)]
        uT = const.tile([128, KH, MACRO], BF16, tag="uT")

        def ln_stats(tag, src):
            """Row stats of src [128, D] f32: returns (nmu, v) = (-mean, var+1e-5)."""
            sqscr = pa.tile([128, D], BF16, tag="sqscr", bufs=1, name="sqscr")
            sqs = pa.tile([128, 1], F32, tag=tag + "_sqs", bufs=2, name="sqs")
            nc.scalar.activation(sqscr[:], src[:], AF.Square, accum_out=sqs[:])
            xsum = pa.tile([128, 1], F32, tag=tag + "_xs", bufs=2, name="xs")
            nc.vector.tensor_reduce(xsum[:], src[:], mybir.AxisListType.X, ALU.add)
            nmu = pa.tile([128, 1], F32, tag=tag + "_nmu", bufs=2, name="nmu")
            nc.vector.tensor_scalar(nmu[:], xsum[:], -1.0 / D, None, op0=ALU.mult)
            v = pa.tile([128, 1], F32, tag=tag + "_v", bufs=2, name="v")
            nc.vector.tensor_scalar(v[:], sqs[:], 1.0 / D, 1e-5, op0=ALU.mult, op1=ALU.add)
            m2 = pa.tile([128, 1], F32, tag=tag + "_m2", bufs=2, name="m2")
            nc.vector.tensor_mul(m2[:], nmu[:], nmu[:])
            nc.vector.tensor_sub(v[:], v[:], m2[:])
            return nmu, v

        def newton_rsqrt(tag, v):
            """y ~ rsqrt(v) for v [128,1] f32 > 0; quake seed + 2 NR iters on DVE."""
            y = pa.tile([128, 1], F32, tag=tag + "_y", bufs=2, name="y")
            a = pa.tile([128, 1], F32, tag=tag + "_a", bufs=2, name="a")
            nc.vector.tensor_scalar(
                y[:].bitcast(I32), v[:].bitcast(I32), 1, -1,
                op0=ALU.logical_shift_right, op1=ALU.bitwise_xor,
            )
            nc.vector.tensor_scalar(
                y[:].bitcast(I32), y[:].bitcast(I32), 0x5F3759E0, None, op0=ALU.add
            )
            for it in range(2):
                nc.vector.tensor_mul(a[:], y[:], y[:])
                nc.vector.tensor_mul(a[:], a[:], v[:])
                nc.vector.tensor_scalar(a[:], a[:], -0.5, 1.5, op0=ALU.mult, op1=ALU.add)
                nc.vector.tensor_mul(y[:], y[:], a[:])
            return y

        # ---- pass-A chunks for one 128-token tile ----
        def c1_stats(t):
            x_sb = xs[t]
            nmu, v = ln_stats("s1", x_sb)
            rstd = newton_rsqrt("n1", v)
            nmr1 = pa.tile([128, 1], F32, tag="nmr1", bufs=2, name="nmr1")
            nc.vector.tensor_mul(nmr1[:], nmu[:], rstd[:])
            hu = pa.tile([128, D], BF16, tag="hu", bufs=2, name="hu")
            nc.vector.tensor_scalar(hu[:], x_sb[:], rstd[:], nmr1[:],
                                    op0=ALU.mult, op1=ALU.add)
            hus[t] = hu

        def c2_lnT(t):
            hu = hus.pop(t)
            trps = psa.tile([128, KD, 128], BF16, tag="trps", bufs=1,
                            padded_shape=[128, 8, 128], name="trps")
            for k in range(KD):
                nc.tensor.transpose(trps[:, k, :], hu[:, ts(k, 128)], idb_sb[:])
            lnT = pa.tile([128, KD, 128], BF16, tag="lnT", bufs=2, name="lnT")
            nc.scalar.copy(lnT[:], trps[:])
            lnts[t] = lnT

        def c3_gv(t):
            lnT = lnts.pop(t)
            pgv = [None] * 3
            for bb in range(3):
                pgv[bb] = psa.tile([128, 512], F32, tag="gvps", bufs=3, name="pgv")
                mm = [(lnT[:, k, :], wgv_sb[:, k, 512 * bb:512 * (bb + 1)])
                      for k in range(KD)]
                if gv_b:
                    mm.append((ones_sb[0:1, 0:128],
                               bgv_sb[0:1, 512 * bb:512 * (bb + 1)]))
                for i, (l, r) in enumerate(mm):
                    nc.tensor.matmul(pgv[bb][:], l, r,
                                     start=(i == 0), stop=(i == len(mm) - 1))
            # gvg packs [g 0:768 | g*v 0:768]
            gvg = pa.tile([128, 2 * D], BF16, tag="gvg", bufs=2, name="gvg")
            nc.scalar.activation(gvg[:, 0:512], pgv[0][:], AF.Sigmoid)
            nc.scalar.activation(gvg[:, 512:768], pgv[2][:, 0:256], AF.Sigmoid)
            nc.vector.tensor_mul(gvg[:, 768:1280], gvg[:, 0:512], pgv[1][:])
            nc.vector.tensor_mul(gvg[:, 1280:1536], gvg[:, 512:768],
                                 pgv[2][:, 256:512])
            return gvg

        def c4a_cs(t, gvg):
            cs = [None] * 3
            for bb in range(3):
                cs[bb] = psa.tile([128, 512], F32, tag="gvps", bufs=3, name="cs")
                nc.tensor.matmul(cs[bb][:], u_sb[:],
                                 gvg[:, 512 * bb:512 * (bb + 1)],
                                 start=True, stop=True)
            den = pa.tile([128, D], F32, tag="den", bufs=1, name="den")
            mem = pa.tile([128, D], F32, tag="mem", bufs=1, name="mem")
            nc.scalar.activation(den[:, 0:512], cs[0][:], AF.Identity,
                                 bias=eps_sb[:])
            nc.scalar.activation(den[:, 512:768], cs[1][:, 0:256], AF.Identity,
                                 bias=eps_sb[:])
            nc.vector.reciprocal_approx_fast(den[:], den[:])
            nc.vector.tensor_mul(mem[:, 0:256], den[:, 0:256], cs[1][:, 256:512])
            nc.vector.tensor_mul(mem[:, 256:768], den[:, 256:768], cs[2][:])
            x_sb = xs.pop(t)
            x2 = pa.tile([128, D], F32, tag="x2", bufs=2 * NS, name="x2")
            nc.vector.tensor_add(x2[:], x_sb[:], mem[:])
            x2s[t] = x2

        def c4b_ln2(t):
            x2 = x2s[t]
            nmu2, v2 = ln_stats("s2", x2)
            rstd2 = newton_rsqrt("n2", v2)
            nmr2 = pa.tile([128, 1], F32, tag="nmr2", bufs=2, name="nmr2")
            nc.vector.tensor_mul(nmr2[:], nmu2[:], rstd2[:])
            h2s = pa.tile([128, D], BF16, tag="h2s", bufs=2, name="h2s")
            nc.vector.tensor_scalar(h2s[:], x2[:], rstd2[:], nmr2[:],
                                    op0=ALU.mult, op1=ALU.add)
            hus[("h2", t)] = h2s

        def c5_h2T(t, h2t_sb, s):
            h2s = hus.pop(("h2", t))
            trps = psa.tile([128, KD, 128], BF16, tag="trps", bufs=1,
                            padded_shape=[128, 8, 128], name="trps")
            for k in range(KD):
                nc.tensor.transpose(trps[:, k, :], h2s[:, ts(k, 128)], idb_sb[:])
            nc.scalar.copy(h2t_sb[:, :, 128 * s:128 * (s + 1)], trps[:])

        def tile_chunks(t, h2t_sb, s):
            gvg_box = []
            return [
                lambda: c1_stats(t),
                lambda: c2_lnT(t),
                lambda: gvg_box.append(c3_gv(t)),
                lambda: c4a_cs(t, gvg_box.pop()),
                lambda: c4b_ln2(t),
                lambda: c5_h2T(t, h2t_sb, s),
            ]

        def macro_chunks(m, h2t_sb):
            """Interleave order for the 4 tiles of macro m: stats for all
            tiles first, then the gv/cumsum chains, transposes last — gives
            every cross-engine dependency >= 1 FFN1 f-block of headroom."""
            percall = [tile_chunks(m * NS + s, h2t_sb, s) for s in range(NS)]
            order = [c[0] for c in percall]
            for c in percall:
                order += c[1:5]
            order += [c[5] for c in percall]
            return order

        # ---- FFN for one macro (512 tokens), with interleaved chunks ----
        def emit_ffn(m, h2t_sb, chunks):
            tok0 = MACRO * m
            ci = iter(chunks)
            # FFN1: f-block stationary -> uT[f] = gelu-ish in [f, tok] layout
            for f in range(KH):
                pT = psa.tile([128, MACRO], F32, tag="pT", bufs=2, name="pT")
                mm = [(w1_sb[:, k, 128 * f:128 * (f + 1)], h2t_sb[:, k, :])
                      for k in range(KD)]
                if f1bias:
                    mm.append((b1_sb[0:1, 128 * f:128 * (f + 1)],
                               ones_sb[0:1, 0:MACRO]))
                for i, (l, r) in enumerate(mm):
                    nc.tensor.matmul(pT[:], l, r,
                                     start=(i == 0), stop=(i == len(mm) - 1))
                e_sb = pa.tile([128, MACRO], BF16, tag="e", bufs=2, name="e_sb")
                nc.scalar.activation(e_sb[:], pT[:],
                                     AF.Erf if erf_ok else AF.Tanh,
                                     scale=INV_SQRT2)
                nc.vector.scalar_tensor_tensor(uT[:, f, :], e_sb[:], 1.0, pT[:],
                                               op0=ALU.add, op1=ALU.mult)
                nxt = next(ci, None)
                if nxt is not None:
                    nxt()
            for nxt in ci:
                nxt()
            # FFN2: token-stationary; output in natural [tok, d] layout.
            # Per token-block: 512-wide half then 256-wide half in separate
            # single-buffered banks so each drain overlaps the other half's
            # (or the next block's) matmuls.
            for tb in range(NS):
                t = m * NS + tb
                x2 = x2s.pop(t)
                last = (m == NM - 1 and tb == NS - 1)
                osb = pa.tile([128, D], F32, tag="osb", bufs=2, name="osb")
                for tag, off, ncols in (("outk0", 0, 512), ("outk1", 512, 256)):
                    ok = psa.tile([128, 512], F32, tag=tag, bufs=1, name=tag)
                    nmm = KH + (1 if f2bias else 0)
                    for f in range(KH):
                        nc.tensor.matmul(ok[:, 0:ncols],
                                         uT[:, f, 128 * tb:128 * (tb + 1)],
                                         w2_sb[:, f, off:off + ncols],
                                         start=(f == 0), stop=(f == nmm - 1))
                    if f2bias:
                        nc.tensor.matmul(ok[:, 0:ncols], ones_sb[0:1, 0:128],
                                         b2_sb[0:1, off:off + ncols],
                                         start=False, stop=True)
                    nc.vector.tensor_add(osb[:, off:off + ncols], ok[:, 0:ncols],
                                         x2[:, off:off + ncols])
                    if last:
                        # tail trim: ship each half as soon as it drains
                        nc.sync.dma_start(
                            out_d[tok0 + 128 * tb:tok0 + 128 * (tb + 1),
                                  off:off + ncols],
                            osb[:, off:off + ncols])
                if not last:
                    nc.sync.dma_start(
                        out_d[tok0 + 128 * tb:tok0 + 128 * (tb + 1), :], osb[:])

        for _ in range(reps):
            # prologue: macro 0 pass A, diagonally pipelined
            chunks0 = [tile_chunks(t, h2t_tiles[0], t) for t in range(NS)]
            for wave in range(6 + 2 * (NS - 1)):
                for s in range(NS):
                    k = wave - 2 * s
                    if 0 <= k < 6:
                        chunks0[s][k]()
            for m in range(NM):
                if m + 1 < NM:
                    if m > 0:  # macro-1 x tiles were prefetched before w2
                        for s in range(NS):
                            stage_xdma((m + 1) * NS + s)
                    nxt_chunks = macro_chunks(m + 1, h2t_tiles[(m + 1) % 2])
                else:
                    nxt_chunks = []
                emit_ffn(m, h2t_tiles[m % 2], nxt_chunks)

    nc.compile()
    return nc


def _fold(inputs):
    f32 = np.float32
    bf16 = ml_dtypes.bfloat16
    n1w = np.asarray(inputs["norm1_w"], f32)
    n1b = np.asarray(inputs["norm1_b"], f32)
    n2w = np.asarray(inputs["norm2_w"], f32)
    n2b = np.asarray(inputs["norm2_b"], f32)
    gW = np.asarray(inputs["gate_W"], f32)
    gb = np.asarray(inputs["gate_b"], f32)
    vW = np.asarray(inputs["value_W"], f32)
    vb = np.asarray(inputs["value_b"], f32)
    W1 = np.asarray(inputs["ffn_W1"], f32)
    b1 = np.asarray(inputs["ffn_b1"], f32)
    W2 = np.asarray(inputs["ffn_W2"], f32)
    b2 = np.asarray(inputs["ffn_b2"], f32)

    bg = (n1b @ gW + gb).astype(bf16).reshape(1, D)
    bv = (n1b @ vW + vb).astype(bf16).reshape(1, D)
    b1f = (n2b @ W1 + b1).astype(bf16).reshape(1, H)
    b2f = b2.astype(bf16).reshape(1, D)
    flags = (bool(bg.any()), bool(bv.any()), bool(b1f.any()), bool(b2f.any()))

    tri = np.triu(np.ones((CHUNK, CHUNK), f32))
    u = np.zeros((128, 128), f32)
    for c in range(128 // CHUNK):
        u[c * CHUNK:(c + 1) * CHUNK, c * CHUNK:(c + 1) * CHUNK] = tri

    gWs = n1w[:, None] * gW
    vWs = n1w[:, None] * vW
    wgv = np.concatenate(
        [gWs[:, 0:512], vWs[:, 0:512], gWs[:, 512:768], vWs[:, 512:768]], axis=1)
    arrs = {
        "wgv": np.ascontiguousarray(
            wgv.reshape(KD, 128, 2 * D).transpose(1, 0, 2).astype(bf16)),
        "w1": np.ascontiguousarray(
            (n2w[:, None] * W1).reshape(KD, 128, H).transpose(1, 0, 2).astype(bf16)),
        "w2": np.ascontiguousarray(
            (0.5 * W2).reshape(KH, 128, D).transpose(1, 0, 2).astype(bf16)),
        "u": u.astype(bf16),
        "idb": np.eye(128, dtype=bf16),
    }
    if flags[0] or flags[1]:
        arrs["bgv"] = np.concatenate(
            [bg[:, 0:512], bv[:, 0:512], bg[:, 512:768], bv[:, 512:768]], axis=1)
    if flags[2]:
        arrs["b1"] = b1f
    if flags[3]:
        arrs["b2"] = b2f
    return arrs, flags


_CACHE: dict = {}


def _get_exec(flags):
    """Build (once) the Bass module and a cached jitted PJRT executable."""
    if _CACHE.get("flags") == flags:
        return _CACHE
    import jax
    from concourse import mybir
    from concourse.bass2jax import (
        Mesh, PartitionSpec, shard_map, _bass_exec_p, install_neuronx_cc_hook,
        partition_id_tensor,
    )

    nc = build(TPC, True, *flags)
    install_neuronx_cc_hook()
    assert nc.dbg_addr is None
    partition_name = nc.partition_id_tensor.name if nc.partition_id_tensor else None

    in_names, out_names, out_avals, zero_outs = [], [], [], []
    for alloc in nc.m.functions[0].allocations:
        if not isinstance(alloc, mybir.MemoryLocationSet):
            continue
        name = alloc.memorylocations[0].name
        if alloc.kind == "ExternalInput":
            if name != partition_name:
                in_names.append(name)
        elif alloc.kind == "ExternalOutput":
            shape = tuple(alloc.tensor_shape)
            dtype = mybir.dt.np(alloc.dtype)
            out_names.append(name)
            out_avals.append(jax.core.ShapedArray(shape, dtype))
            zero_outs.append(np.zeros(shape, dtype))
    n_params = len(in_names)
    n_outs = len(out_avals)
    all_names = in_names + out_names
    if partition_name is not None:
        all_names = all_names + [partition_name]
    donate = tuple(range(n_params, n_params + n_outs))

    def _body(*args):
        operands = list(args)
        if partition_name is not None:
            operands.append(partition_id_tensor())
        outs = _bass_exec_p.bind(
            *operands,
            out_avals=tuple(out_avals),
            in_names=tuple(all_names),
            out_names=tuple(out_names),
            lowering_input_output_aliases=(),
            sim_require_finite=True,
            sim_require_nnan=True,
            nc=nc,
        )
        return tuple(outs)

    devices = jax.devices()[:NCORES]
    assert len(devices) == NCORES
    mesh = Mesh(np.asarray(devices), ("core",))
    sharded = jax.jit(
        shard_map(_body, mesh=mesh, in_specs=(PartitionSpec("core"),) * (n_params + n_outs),
                  out_specs=(PartitionSpec("core"),) * n_outs, check_rep=False),
        donate_argnums=donate, keep_unused=True,
    )
    _CACHE.clear()
    _CACHE.update(
        flags=flags, nc=nc, sharded=sharded, in_names=in_names,
        out_names=out_names, out_avals=out_avals, zero_outs=zero_outs, mesh=mesh,
    )
    return _CACHE


def _run(arrs, flags, x_flat):
    st = _get_exec(flags)
    concat_in = []
    for name in st["in_names"]:
        if name == "x":
            concat_in.append(np.ascontiguousarray(x_flat))
        else:
            a = arrs[name]
            concat_in.append(np.concatenate([a] * NCORES, axis=0))
    concat_zeros = [
        np.zeros((NCORES * z.shape[0], *z.shape[1:]), z.dtype) for z in st["zero_outs"]
    ]
    out_arrs = st["sharded"](*concat_in, *concat_zeros)
    i = st["out_names"].index("out")
    return np.asarray(out_arrs[i])


def _assemble(results):
    """Full [B,S,D] output from per-core result dicts."""
    parts = [np.asarray(results[c]["out"]) for c in range(NCORES)]
    return np.concatenate(parts, axis=0).reshape(B, S, D).astype(np.float32)


def kernel(**inputs):
    x = np.asarray(inputs["x"], np.float32).reshape(TOTAL, D)
    arrs, flags = _fold(inputs)
    try:
        o = _run(arrs, flags, x)
        return np.asarray(o).reshape(B, S, D).astype(np.float32)
    except Exception:
        from concourse.bass_utils import run_bass_kernel_spmd
        if _CACHE.get("flags") != flags or "nc" not in _CACHE:
            _CACHE.clear()
            _CACHE["nc"] = build(TPC, True, *flags)
            _CACHE["flags"] = flags
        in_maps = [
            {**arrs, "x": np.ascontiguousarray(x[c * TPC:(c + 1) * TPC])}
            for c in range(NCORES)
        ]
        res = run_bass_kernel_spmd(_CACHE["nc"], in_maps, list(range(NCORES)),
                                   trace=False)
        return _assemble(res.results)
